# revision 5
# baseline (speedup 1.0000x reference)
"""MinGRU forward on 8 Trainium2 NeuronCores.

Math (per batch b, hidden unit j, time t):
    k  = x @ Wz.T + bz;  tilde = x @ Wh.T + bh
    z  = sigmoid(k);     a = 1 - z = sigmoid(-k)
    g  = where(tilde >= 0, tilde + 0.5, sigmoid(tilde))
       = relu(tilde) + sigmoid(min(tilde, 0))
    h[t] = a[t] * h[t-1] + z[t] * g[t]        (h[-1] = 0)

The reference evaluates the recurrence as a log-space parallel scan; here we
run it directly in linear space with the DVE TensorTensorScan instruction
(fp32 state), which is numerically benign because a in (0,1) and z*g is O(1).

Sharding: data-parallel over batch (B=8 -> one batch element per core),
weights replicated. Per core the kernel computes in [H, T] layout so the
time axis lands on the SBUF free dimension (scan direction); the host
pre-transposes x -> xT per batch and transposes the [H, T] output back.
"""

import numpy as np

import concourse.bass as bass
import concourse.mybir as mybir
from concourse import bacc, tile
from concourse.bass_utils import run_bass_kernel_spmd

P = 128          # SBUF partitions
B, T, D, H = 8, 4096, 1024, 1024
TC = 512         # time-chunk (fp32 moving-operand max / one PSUM bank)
NKB = D // P     # contraction blocks
NHB = H // P     # hidden blocks
NCH = T // TC    # time chunks

FP32 = mybir.dt.float32
FP32R = mybir.dt.float32r   # full-rate PE matmul for 4-byte data at N>=256
AO = mybir.AluOpType
AF = mybir.ActivationFunctionType


def build_module():
    nc = bacc.Bacc("TRN2", target_bir_lowering=False, debug=False,
                   num_devices=B)
    xT = nc.dram_tensor("xT", [D, T], FP32R, kind="ExternalInput")
    wzT = nc.dram_tensor("wzT", [D, H], FP32R, kind="ExternalInput")
    whT = nc.dram_tensor("whT", [D, H], FP32R, kind="ExternalInput")
    bz = nc.dram_tensor("bz", [H], FP32, kind="ExternalInput")
    bh = nc.dram_tensor("bh", [H], FP32, kind="ExternalInput")
    out = nc.dram_tensor("out_hT", [H, T], FP32, kind="ExternalOutput")

    with tile.TileContext(nc) as tc:
        with (
            tc.tile_pool(name="wpool", bufs=1) as wpool,
            tc.tile_pool(name="xpool", bufs=2) as xpool,
            tc.tile_pool(name="psum", bufs=3, space="PSUM") as psum_pool,
            tc.tile_pool(name="ew", bufs=3) as ew,
            tc.tile_pool(name="hout", bufs=3) as hpool,
            tc.tile_pool(name="misc", bufs=1) as misc,
        ):
            # Weights, transposed on host to [D, H]: partition = d % 128.
            wz_sb = wpool.tile([P, NKB, H], FP32R, tag="wz")
            wh_sb = wpool.tile([P, NKB, H], FP32R, tag="wh")
            nc.sync.dma_start(wz_sb[:], wzT.rearrange("(kb p) h -> p kb h", p=P))
            nc.sync.dma_start(wh_sb[:], whT.rearrange("(kb p) h -> p kb h", p=P))

            # Biases as per-partition columns: bias[hb*128 + p] -> [p, hb].
            bz_sb = misc.tile([P, NHB], FP32, tag="bz")
            bh_sb = misc.tile([P, NHB], FP32, tag="bh")
            nc.sync.dma_start(bz_sb[:], bz.rearrange("(hb p) -> p hb", p=P))
            nc.sync.dma_start(bh_sb[:], bh.rearrange("(hb p) -> p hb", p=P))
            nbz_sb = misc.tile([P, NHB], FP32, tag="nbz")
            nc.scalar.mul(nbz_sb[:], bz_sb[:], -1.0)

            # Per-hidden-block scan carry, chained across time chunks.
            carries = [misc.tile([P, 1], FP32, tag=f"carry{hb}", name=f"carry{hb}")
                       for hb in range(NHB)]

            xT_r = xT.rearrange("(kb p) t -> p kb t", p=P)
            for c in range(NCH):
                xt = xpool.tile([P, NKB, TC], FP32R, tag="xt")
                nc.sync.dma_start(xt[:], xT_r[:, :, c * TC:(c + 1) * TC])
                for hb in range(NHB):
                    pk = psum_pool.tile([P, TC], FP32, tag="pk")
                    pt = psum_pool.tile([P, TC], FP32, tag="pt")
                    for kb in range(NKB):
                        nc.tensor.matmul(
                            pk[:], wz_sb[:, kb, hb * P:(hb + 1) * P], xt[:, kb, :],
                            start=(kb == 0), stop=(kb == NKB - 1),
                        )
                    for kb in range(NKB):
                        nc.tensor.matmul(
                            pt[:], wh_sb[:, kb, hb * P:(hb + 1) * P], xt[:, kb, :],
                            start=(kb == 0), stop=(kb == NKB - 1),
                        )
                    # a = sigmoid(-(k + bz))
                    a = ew.tile([P, TC], FP32, tag="a")
                    nc.scalar.activation(a[:], pk[:], AF.Sigmoid,
                                         bias=nbz_sb[:, hb:hb + 1], scale=-1.0)
                    # z = 1 - a
                    z = ew.tile([P, TC], FP32, tag="z")
                    nc.vector.tensor_scalar(z[:], a[:], -1.0, 1.0, AO.mult, AO.add)
                    # tm = min(tilde, 0);  rp = relu(tilde)   (tilde = pt + bh)
                    tm = ew.tile([P, TC], FP32, tag="tm")
                    nc.vector.tensor_scalar(tm[:], pt[:], bh_sb[:, hb:hb + 1], 0.0,
                                            AO.add, AO.min)
                    rp = ew.tile([P, TC], FP32, tag="rp")
                    nc.vector.tensor_scalar(rp[:], pt[:], bh_sb[:, hb:hb + 1], 0.0,
                                            AO.add, AO.max)
                    # g = relu(tilde) + sigmoid(min(tilde, 0));  b = z * g
                    s = ew.tile([P, TC], FP32, tag="s")
                    nc.scalar.activation(s[:], tm[:], AF.Sigmoid)
                    g = ew.tile([P, TC], FP32, tag="g")
                    nc.vector.tensor_add(g[:], rp[:], s[:])
                    bb = ew.tile([P, TC], FP32, tag="bb")
                    nc.vector.tensor_mul(bb[:], z[:], g[:])
                    # h[t] = a[t] * h[t-1] + b[t]
                    h = hpool.tile([P, TC], FP32, tag="h")
                    init = 0.0 if c == 0 else carries[hb][:]
                    nc.vector.tensor_tensor_scan(h[:], a[:], bb[:], init,
                                                 AO.mult, AO.add)
                    if c < NCH - 1:
                        nc.gpsimd.tensor_copy(carries[hb][:], h[:, TC - 1:TC])
                    nc.sync.dma_start(out[hb * P:(hb + 1) * P, c * TC:(c + 1) * TC],
                                      h[:])
    nc.compile()
    return nc


_NC_CACHE = None


def _get_module():
    global _NC_CACHE
    if _NC_CACHE is None:
        _NC_CACHE = build_module()
    return _NC_CACHE


def _run(inputs, trace=False, **kw):
    x = np.asarray(inputs["x"], dtype=np.float32)
    wzT = np.ascontiguousarray(np.asarray(inputs["Wz"], dtype=np.float32).T)
    whT = np.ascontiguousarray(np.asarray(inputs["Wh"], dtype=np.float32).T)
    bz = np.asarray(inputs["bz"], dtype=np.float32)
    bh = np.asarray(inputs["bh"], dtype=np.float32)

    in_maps = []
    for b in range(B):
        in_maps.append({
            "xT": np.ascontiguousarray(x[b].T),
            "wzT": wzT,
            "whT": whT,
            "bz": bz,
            "bh": bh,
        })
    nc = _get_module()
    res = run_bass_kernel_spmd(nc, in_maps, list(range(B)), trace=trace, **kw)
    out = np.stack([res.results[b]["out_hT"].T for b in range(B)])
    return out, res


def kernel(**inputs) -> np.ndarray:
    out, _ = _run(inputs, trace=False)
    return out


# revision 9
# speedup vs baseline: 1.0492x; 1.0492x over previous
"""MinGRU forward on 8 Trainium2 NeuronCores.

Math (per batch b, hidden unit j, time t):
    k  = x @ Wz.T + bz;  tilde = x @ Wh.T + bh
    z  = sigmoid(k);     a = 1 - z = sigmoid(-k)
    g  = where(tilde >= 0, tilde + 0.5, sigmoid(tilde))
       = relu(tilde) + sigmoid(min(tilde, 0))
    h[t] = a[t] * h[t-1] + z[t] * g[t]        (h[-1] = 0)

The reference evaluates the recurrence as a log-space parallel scan; here we
run it directly in linear space with the DVE TensorTensorScan instruction
(fp32 state), which is numerically benign because a in (0,1) and z*g is O(1).

Sharding: data-parallel over batch (B=8 -> one batch element per core),
weights replicated. Per core the kernel computes in [H, T] layout so the
time axis lands on the SBUF free dimension (scan direction); the host
pre-transposes x -> xT per batch and transposes the [H, T] output back.
"""

import numpy as np

import concourse.bass as bass
import concourse.mybir as mybir
from concourse import bacc, tile
from concourse.bass_utils import run_bass_kernel_spmd

P = 128          # SBUF partitions
B, T, D, H = 8, 4096, 1024, 1024
TC = 512         # time-chunk (fp32 moving-operand max / one PSUM bank)
NKB = D // P     # contraction blocks
NHB = H // P     # hidden blocks
NCH = T // TC    # time chunks

FP32 = mybir.dt.float32
FP32R = mybir.dt.float32r   # full-rate PE matmul for 4-byte data at N>=256
AO = mybir.AluOpType
AF = mybir.ActivationFunctionType


def build_module():
    nc = bacc.Bacc("TRN2", target_bir_lowering=False, debug=False,
                   num_devices=B)
    xT = nc.dram_tensor("xT", [D, T], FP32R, kind="ExternalInput")
    wzT = nc.dram_tensor("wzT", [D, H], FP32R, kind="ExternalInput")
    whT = nc.dram_tensor("whT", [D, H], FP32R, kind="ExternalInput")
    bz = nc.dram_tensor("bz", [H], FP32, kind="ExternalInput")
    bh = nc.dram_tensor("bh", [H], FP32, kind="ExternalInput")
    out = nc.dram_tensor("out_hT", [H, T], FP32, kind="ExternalOutput")

    with tile.TileContext(nc) as tc:
        with (
            tc.tile_pool(name="wpool", bufs=1) as wpool,
            tc.tile_pool(name="xpool", bufs=3) as xpool,
            tc.tile_pool(name="psum", bufs=3, space="PSUM") as psum_pool,
            tc.tile_pool(name="ew", bufs=3) as ew,
            tc.tile_pool(name="hout", bufs=3) as hpool,
            tc.tile_pool(name="misc", bufs=1) as misc,
        ):
            # Weights, transposed on host to [D, H]: partition = d % 128.
            wz_sb = wpool.tile([P, NKB, H], FP32R, tag="wz")
            wh_sb = wpool.tile([P, NKB, H], FP32R, tag="wh")
            nc.sync.dma_start(wz_sb[:], wzT.rearrange("(kb p) h -> p kb h", p=P))
            nc.sync.dma_start(wh_sb[:], whT.rearrange("(kb p) h -> p kb h", p=P))

            # Biases as per-partition columns: bias[hb*128 + p] -> [p, hb].
            bz_sb = misc.tile([P, NHB], FP32, tag="bz")
            bh_sb = misc.tile([P, NHB], FP32, tag="bh")
            nc.sync.dma_start(bz_sb[:], bz.rearrange("(hb p) -> p hb", p=P))
            nc.sync.dma_start(bh_sb[:], bh.rearrange("(hb p) -> p hb", p=P))
            nbz_sb = misc.tile([P, NHB], FP32, tag="nbz")
            nc.scalar.mul(nbz_sb[:], bz_sb[:], -1.0)
            nbh_sb = misc.tile([P, NHB], FP32, tag="nbh")
            nc.scalar.mul(nbh_sb[:], bh_sb[:], -1.0)

            # Per-hidden-block scan carry, chained across time chunks.
            carries = [misc.tile([P, 1], FP32, tag=f"carry{hb}", name=f"carry{hb}")
                       for hb in range(NHB)]

            xT_r = xT.rearrange("(kb p) t -> p kb t", p=P)

            def load_chunk(c):
                xt = xpool.tile([P, NKB, TC], FP32R, tag="xt", name=f"xt{c}")
                nc.sync.dma_start(xt[:], xT_r[:, :, c * TC:(c + 1) * TC])
                return xt

            # Software-pipelined input prefetch: issue chunk c+1's load before
            # chunk c's compute/output DMAs enter the queues.
            xt = load_chunk(0)
            for c in range(NCH):
                xt_next = load_chunk(c + 1) if c + 1 < NCH else None
                for hb in range(NHB):
                    pk = psum_pool.tile([P, TC], FP32, tag="pk")
                    pt = psum_pool.tile([P, TC], FP32, tag="pt")
                    for kb in range(NKB):
                        nc.tensor.matmul(
                            pk[:], wz_sb[:, kb, hb * P:(hb + 1) * P], xt[:, kb, :],
                            start=(kb == 0), stop=(kb == NKB - 1),
                        )
                    for kb in range(NKB):
                        nc.tensor.matmul(
                            pt[:], wh_sb[:, kb, hb * P:(hb + 1) * P], xt[:, kb, :],
                            start=(kb == 0), stop=(kb == NKB - 1),
                        )
                    # a = sigmoid(-(k + bz))
                    a = ew.tile([P, TC], FP32, tag="a")
                    nc.scalar.activation(a[:], pk[:], AF.Sigmoid,
                                         bias=nbz_sb[:, hb:hb + 1], scale=-1.0)
                    # rn = relu(-tilde);  s = sigmoid(-rn) = sigmoid(min(tilde,0))
                    rn = ew.tile([P, TC], FP32, tag="rn")
                    nc.scalar.activation(rn[:], pt[:], AF.Relu,
                                         bias=nbh_sb[:, hb:hb + 1], scale=-1.0)
                    s = ew.tile([P, TC], FP32, tag="s")
                    nc.scalar.activation(s[:], rn[:], AF.Sigmoid, scale=-1.0)
                    # rp = relu(tilde)
                    rp = ew.tile([P, TC], FP32, tag="rp")
                    nc.scalar.activation(rp[:], pt[:], AF.Relu,
                                         bias=bh_sb[:, hb:hb + 1])
                    # z = 1 - a;  g = rp + s;  b = z * g
                    z = ew.tile([P, TC], FP32, tag="z")
                    nc.vector.tensor_scalar(z[:], a[:], -1.0, 1.0, AO.mult, AO.add)
                    g = ew.tile([P, TC], FP32, tag="g")
                    nc.vector.tensor_add(g[:], rp[:], s[:])
                    bb = ew.tile([P, TC], FP32, tag="bb")
                    nc.vector.tensor_mul(bb[:], z[:], g[:])
                    # h[t] = a[t] * h[t-1] + b[t]
                    h = hpool.tile([P, TC], FP32, tag="h")
                    init = 0.0 if c == 0 else carries[hb][:]
                    nc.vector.tensor_tensor_scan(h[:], a[:], bb[:], init,
                                                 AO.mult, AO.add)
                    if c < NCH - 1:
                        nc.gpsimd.tensor_copy(carries[hb][:], h[:, TC - 1:TC])
                    # Output DMA on the Activation HWDGE queue so stores never
                    # block the next chunk's input prefetch on the SP queue.
                    nc.scalar.dma_start(out[hb * P:(hb + 1) * P,
                                            c * TC:(c + 1) * TC], h[:])
                xt = xt_next
    nc.compile()
    return nc


_NC_CACHE = None


def _get_module():
    global _NC_CACHE
    if _NC_CACHE is None:
        _NC_CACHE = build_module()
    return _NC_CACHE


def _run(inputs, trace=False, **kw):
    x = np.asarray(inputs["x"], dtype=np.float32)
    wzT = np.ascontiguousarray(np.asarray(inputs["Wz"], dtype=np.float32).T)
    whT = np.ascontiguousarray(np.asarray(inputs["Wh"], dtype=np.float32).T)
    bz = np.asarray(inputs["bz"], dtype=np.float32)
    bh = np.asarray(inputs["bh"], dtype=np.float32)

    in_maps = []
    for b in range(B):
        in_maps.append({
            "xT": np.ascontiguousarray(x[b].T),
            "wzT": wzT,
            "whT": whT,
            "bz": bz,
            "bh": bh,
        })
    nc = _get_module()
    res = run_bass_kernel_spmd(nc, in_maps, list(range(B)), trace=trace, **kw)
    out = np.stack([res.results[b]["out_hT"].T for b in range(B)])
    return out, res


def kernel(**inputs) -> np.ndarray:
    out, _ = _run(inputs, trace=False)
    return out
